# revision 110
# baseline (speedup 1.0000x reference)
"""AVWGCN Trainium2 kernel: adaptive-adjacency Chebyshev GCN.

Math (per core, batch-sharded over B: 8 batches/core):
  A = relu(E @ E^T) (symmetric), M = exp(A), r = rowsum(M), S = diag(1/r) M
  X2[m,(b,c)] = x[b,m,c]
  x1 = diag(1/r) (M @ X2)            (T1 term)
  x2o = diag(1/r) (M @ x1)           (= S^2 x; T2 = 2 S^2 - I folded on host)
  out[b,n,o] = sum_d E[n,d] * ( sum_{k,i} xg_k[n,(b,i)] Wp'[d,k,i,o] + bp[d,o] )
    with Wp'_0 = Wp_0 - Wp_2, Wp'_2 = 2 Wp_2 (host fold of the Chebyshev -x
    term), so xg = [x, S x, S^2 x].
Key structure:
  - stage 1 computes M COLUMN-block by column-block into per-column tiles:
    column j is exactly the lhsT set s3(j)/s4(j) contract with, so the
    ACT-heavy exp pipeline overlaps s3's PE-heavy matmuls with clean
    per-tile dependencies (one stage-1 column finishes COL_LAG iterations
    ahead of its s3 use; steps are interleaved one per s3 matmul to avoid
    head-of-line blocking of the in-order PE queue).
  - row sums ride s3's h0 matmul as an extra ones column of the rhs
    (M @ [X2|1]); reciprocal per column feeds both halves' scales.
  - x1/x2o written interleaved into a padded tile xp[j][n, (b,128)] (x1 at
    cols b*128+c, x2o at b*128+64+c); ONE hw DMA transpose per (half, j)
    lands [x1^T; x2^T] stacked on 128 partitions = the K=128 lhsT (k1;k2).
  - gconv via Z-form: Z[n,(o,d)] = bf16 [x1;x2]^T-pass + ONE fp8 DoubleRow
    pass (0.5 cyc/col) carrying the k0 term exactly via a host-side
    e4m3 hi/lo split (xhi*W0hi + xlo*W0hi + xhi*W0lo; the dropped lo*lo
    term is ~0.1% of the k0 term) with bias on a ones row; epilogue: ACT
    casts Z->bf16, DVE multiplies by broadcast E and tree-reduces d.
  - batch dim processed in two halves; the last DEFER0 epilogues of half 0
    drain inside half 1's s3 phase (whose PE-bound window otherwise leaves
    ACT/DVE idle).
"""

from contextlib import ExitStack

import numpy as np

import concourse.bass as bass
import concourse.mybir as mybir
import concourse.tile as tile
from concourse.bass_utils import run_bass_kernel_spmd

B, N, C, CHEB_K, EMBED = 64, 2048, 64, 3, 16
NCORES = 8
BC = B // NCORES            # batches per core
F = BC * C                  # 512: free width of X2 [m, (b,c)]
FH = F // 2                 # 256: per-batch-half width
NT = N // 128               # 16 n-chunks
FP32 = mybir.dt.float32
BF16 = mybir.dt.bfloat16
FP8 = mybir.dt.float8e4
MM_DT = BF16
DO = C * EMBED              # 1024, Z free width, (o, d) ordered


_WAIT_CAP = {"InstDMACopy": 1}
_WAIT_SAFE = {"InstEventSemaphore", "InstCall",
              "InstUnconditionalBranch", "InstISA", "InstRegisterMove"}


def _split_excess_waits(nc):
    """Walrus rejects compute instructions carrying more sync waits than the
    ISA struct can encode. Hoist excess waits onto an inserted same-engine
    Drain immediately before the instruction (semantically identical)."""
    SyncInfo = None
    n_fix = 0
    for f in nc.m.functions:
        for blk in f.blocks:
            out_insts = []
            for inst in blk.instructions:
                tn = type(inst).__name__
                si = inst.sync_info
                w = list(si.on_wait) if (si is not None and si.on_wait) else []
                cap = _WAIT_CAP.get(tn, 1)
                if tn not in _WAIT_SAFE and len(w) > cap:
                    if SyncInfo is None:
                        SyncInfo = type(si)
                    for wx in w:
                        d = mybir.InstDrain(name=f"I-wsplit{nc.next_id()}",
                                            ins=[], outs=[])
                        d.engine = inst.engine
                        d.sync_info = SyncInfo(on_wait=[wx], on_update=[])
                        out_insts.append(d)
                    si.on_wait = []
                    n_fix += 1
                out_insts.append(inst)
            blk.instructions[:] = out_insts
    return n_fix


def build_nc():
    nc = bass.Bass()
    x2a = nc.dram_tensor("x2a", [N, FH], BF16, kind="ExternalInput").ap()
    x2b = nc.dram_tensor("x2b", [N, FH], BF16, kind="ExternalInput").ap()
    xf8_d = nc.dram_tensor("xf8", [128, 2 * BC * N], FP8, kind="ExternalInput").ap()
    et = nc.dram_tensor("et", [3 * EMBED, N], BF16, kind="ExternalInput").ap()
    etlo_d = nc.dram_tensor("etlo", [3 * EMBED, N], BF16, kind="ExternalInput").ap()
    en = nc.dram_tensor("en", [N, EMBED], BF16, kind="ExternalInput").ap()
    wpfa = nc.dram_tensor("wpfa", [128, DO], BF16, kind="ExternalInput").ap()
    w8_d = nc.dram_tensor("w8", [128, 2 * DO], FP8, kind="ExternalInput").ap()
    # device out is [n, b, c] (matching the SBUF epilogue layout): the DMA
    # gets 512B full-rate runs and 4x fewer descriptors; host transposes
    out = nc.dram_tensor("out", [N, BC, C], BF16, kind="ExternalOutput").ap()

    with tile.TileContext(nc) as tc:
        with ExitStack() as ctx:
            kernel_body(ctx, tc, out, x2a, x2b, xf8_d, et, etlo_d, en, wpfa, w8_d)
    _split_excess_waits(nc)
    return nc


def kernel_body(ctx, tc, out, x2a, x2b, xf8_d, et, etlo_d, en, wpfa, w8_d):
    nc = tc.nc

    singles = ctx.enter_context(tc.tile_pool(name="singles", bufs=1))
    zsb_pool = ctx.enter_context(tc.tile_pool(name="zsb", bufs=3))
    outs_pool = ctx.enter_context(tc.tile_pool(name="outs", bufs=2))

    # ---- constants / inputs ----
    # split-precision E^T, K-stacked: one K=48 matmul computes
    # EhiEhi^T + EhiElo^T + EloEhi^T (lhsT=[Ehi;Ehi;Elo], rhs=[Ehi;Elo;Ehi])
    ethi = singles.tile([3 * EMBED, N], MM_DT, tag="ethi")
    etlo = singles.tile([3 * EMBED, N], MM_DT, tag="etlo")
    # split first chunks so the first stage-1 matmul starts ASAP
    nc.sync.dma_start(out=ethi, in_=et)
    nc.sync.dma_start(out=etlo[:, 0:512], in_=etlo_d[:, 0:512])
    nc.sync.dma_start(out=etlo[:, 512:N], in_=etlo_d[:, 512:N])
    wa_sb = singles.tile([128, DO], MM_DT, tag="wa")   # rows: [k1; 2*k2]
    # fp8 DoubleRow operands for the k0+bias pass (host-prepared hi/lo split):
    #   k-tile 0 rows = [xhi(64); xlo(64)], k-tile 1 rows = [xhi(64); ones; 0]
    #   paired with   [W0hi; W0hi]         and          [W0lo; bias; junk]
    xf8 = singles.tile([128, 2, BC * N], FP8, tag="xf8")
    w8_sb = singles.tile([128, 2, DO], FP8, tag="w8")
    # E chunks for the epilogue: en_sb[p, j, d] = E[j*128+p, d]
    en_sb = singles.tile([128, NT, EMBED], MM_DT, tag="en_sb")
    # x12t: [x1^T; x2^T] stacked on partitions, filled by DMA transposes
    x12t = singles.tile([128, BC * N], MM_DT, tag="x12t")
    x12t_v = x12t.rearrange("p (b n) -> p b n", b=BC)

    # ---- stage 1: M = exp(relu(E E^T)), computed COLUMN-block by
    # column-block into per-column tiles: column j is exactly the lhsT set
    # s3(j)/s4(j) contract with, so each s3(j) depends on one tile that was
    # finished COL_LAG iterations earlier — no cross-phase false deps, no
    # transposes, and stage 1's ACT-heavy exp overlaps s3's PE-heavy matmuls.
    m_col = [singles.tile([128, NT, 128], MM_DT, name=f"mc{j}", tag=f"mc{j}")
             for j in range(NT)]

    def m_ap(a, j):
        """lhsT for contraction chunk a, output chunk j: M[a-rows, j-cols]."""
        return m_col[j][:, a, :]

    ps_mm = ctx.enter_context(tc.tile_pool(name="ps_mm", bufs=2, space="PSUM"))
    ps_z = ctx.enter_context(tc.tile_pool(name="ps_z", bufs=3, space="PSUM"))

    def s1_col_steps(j):
        """Closures (one per matmul) for column-block j of stage 1:
        M[:, jsl] = max(exp(E E^T), 1) in two 8-row-chunk groups. Interleaved
        between s3 matmuls so the PSUM-ring wait on the ACT exp never
        head-of-line-blocks the in-order PE queue."""
        jsl = slice(j * 128, (j + 1) * 128)
        state = {}

        def step(g, i):
            a = g * 8 + i

            def run():
                if i == 0:
                    state["pa"] = ps_z.tile([128, DO], FP32, name="pa", tag="pz")
                pa = state["pa"]
                nc.tensor.matmul(pa[:, i * 128:(i + 1) * 128],
                                 lhsT=ethi[:, a * 128:(a + 1) * 128],
                                 rhs=etlo[:, jsl], start=True, stop=True)
                if i == 7:
                    dst = m_col[j][:, g * 8:(g + 1) * 8, :]
                    nc.scalar.activation(
                        out=dst, in_=pa,
                        func=mybir.ActivationFunctionType.Exp)
                    nc.vector.tensor_scalar_max(dst, dst, 1.0)
            return run

        return [step(g, i) for g in range(2) for i in range(8)]

    # ---- stage 2: X2 halves (h0 includes a ones column at FH for the row
    # sums; see s3), split into two tiles (a 0-7 / 8-15) so the chunked
    # loads have no same-tile WAW chain.
    FW = 2 * FH + 1             # 513: [h0 | ones col | h1]
    x2big2 = [singles.tile([128, NT // 2, FW], MM_DT, name=f"x2big{g}",
                           tag=f"x2big{g}")
              for g in range(2)]
    x2ar = x2a.rearrange("(a p) f -> p a f", a=NT)
    x2br = x2b.rearrange("(a p) f -> p a f", a=NT)
    for g in range(2):
        asl = slice(g * 8, (g + 1) * 8)
        nc.vector.memset(x2big2[g][:, :, FH:FH + 1], 1.0)
        nc.gpsimd.dma_start(out=x2big2[g][:, :, 0:FH], in_=x2ar[:, asl, :])
        nc.gpsimd.dma_start(out=x2big2[g][:, :, FH + 1:FW],
                            in_=x2br[:, asl, :])

    def x2_ap(a, hsl):
        return x2big2[a // 8][:, a % 8, hsl]
    def load_s6_inputs():
        """Issued after the h0 s3 loop: these are first used ~halfway in,
        and issuing them early would contend the (exclusive) DMA engine
        device against the latency-critical mirror transposes."""
        nc.gpsimd.dma_start(out=wa_sb, in_=wpfa)
        xf8v = xf8.rearrange("p t n -> p (t n)")
        for g in range(8):
            csl = slice(g * 4096, (g + 1) * 4096)
            nc.gpsimd.dma_start(out=xf8v[:, csl], in_=xf8_d[:, csl])
        nc.gpsimd.dma_start(out=w8_sb.rearrange("p t n -> p (t n)"), in_=w8_d)
        nc.gpsimd.dma_start(out=en_sb,
                            in_=en.rearrange("(j p) d -> p j d", j=NT))
    rinv = singles.tile([128, NT], FP32, tag="rinv")

    # xp[j]: padded per-half [n, (b4, 128)]: x1 at cols b*128+c, x2o at +64
    xp = [singles.tile([128, 4, 128], MM_DT, name=f"xp{j}", tag=f"xp{j}")
          for j in range(NT)]

    def s3_j(h, j, fill=()):
        """x1[:, h-half] = diag(1/r) M X2[:, h-half] -> xp[j] cols b*128+c.
        `fill` steps (stage-1 chunks) are issued one per s3 matmul."""
        jsl = slice(j * 128, (j + 1) * 128)
        w = FH + 1 if h == 0 else FH
        hsl = slice(0, FH + 1) if h == 0 else slice(FH + 1, FW)
        fill = list(fill)
        pm = ps_mm.tile([128, FH + 1], FP32, tag="pm")
        for a in range(NT):
            nc.tensor.matmul(pm[:, 0:w], lhsT=m_ap(a, j),
                             rhs=x2_ap(a, hsl),
                             start=(a == 0), stop=(a == NT - 1))
            if fill:
                fill.pop(0)()
        if h == 0:
            # rsum rode along as the last column (M @ ones)
            nc.vector.reciprocal(out=rinv[:, j:j + 1], in_=pm[:, FH:FH + 1])
        nc.scalar.activation(out=xp[j][:, :, 0:C], in_=pm[:, 0:FH],
                             func=mybir.ActivationFunctionType.Copy,
                             scale=rinv[:, j:j + 1])
        for f in fill:
            f()

    def s4_j(h, j):
        """x2o = diag(1/r) M x1 -> xp[j] cols b*128+64+c, then transpose."""
        jsl = slice(j * 128, (j + 1) * 128)
        pm = ps_mm.tile([128, FH + 1], FP32, tag="pm")
        for a in range(NT):
            nc.tensor.matmul(pm[:, 0:FH], lhsT=m_ap(a, j),
                             rhs=xp[a][:, :, 0:C],
                             start=(a == 0), stop=(a == NT - 1))
        nc.scalar.activation(out=xp[j][:, :, C:128], in_=pm[:, 0:FH],
                             func=mybir.ActivationFunctionType.Copy,
                             scale=rinv[:, j:j + 1])
        # one XBAR transpose: 4 blocks of 128 cols -> [x1^T; x2^T] on 128
        # partitions, into the 4 b-column ranges of x12t for this half
        nc.sync.dma_start_transpose(
            out=x12t_v[:, h * 4:(h + 1) * 4, jsl],
            in_=xp[j].rearrange("p b q -> p (b q)"))

    def s6_j(h, j, tail=False, pair=False):
        """Z matmuls + epilogue for the 4 batches of half h, chunk j."""
        jsl = slice(j * 128, (j + 1) * 128)
        # E[jsl] broadcast over (4 batches, C outputs) via zero-stride dims
        erep_bc = bass.AP(tensor=en_sb.tensor, offset=en_sb.offset + j * EMBED,
                          ap=[en_sb.ap[0], [0, 4], [0, C], [1, EMBED]])

        zsb = zsb_pool.tile([128, 4, C, EMBED], MM_DT, tag="zsb")
        outt = outs_pool.tile([128, 4, C], MM_DT, tag="outt")
        for bq in range(4):
            b = h * 4 + bq
            col = b * N + j * 128
            pz = ps_z.tile([128, DO], FP32, tag="pz")
            for hh in range(2):
                sl = slice(hh * 512, (hh + 1) * 512)
                nc.tensor.matmul(pz[:, sl], lhsT=x12t[:, col:col + 128],
                                 rhs=wa_sb[:, sl], start=True, stop=False)
                nc.tensor.matmul(pz[:, sl], lhsT=xf8[:, :, col:col + 128],
                                 rhs=w8_sb[:, :, sl], start=False, stop=True,
                                 perf_mode=mybir.MatmulPerfMode.DoubleRow)
            if pair and bq == 0:
                nc.gpsimd.tensor_copy(
                    out=zsb[:, bq].rearrange("p o d -> p (o d)"), in_=pz)
            else:
                nc.scalar.copy(
                    out=zsb[:, bq].rearrange("p o d -> p (o d)"), in_=pz)
            if tail:
                zv = zsb[:, bq:bq + 1]
                ebc = bass.AP(tensor=erep_bc.tensor, offset=erep_bc.offset,
                              ap=[erep_bc.ap[0], [0, 1], [0, C], [1, EMBED]])
                nc.vector.tensor_mul(zv, zv, ebc)
                _epilogue_tree(zsb, outt, h, j, jsl, bq)
        if not tail:
            nc.vector.tensor_mul(zsb, zsb, erep_bc)  # in-place *E (bf16 2x)
            _epilogue_tree(zsb, outt, h, j, jsl, None)

    def _epilogue_tree(zsb, outt, h, j, jsl, bq):
        """d-tree-reduce (DVE for the big levels, Pool for the small) and
        the out DMA; bq=None batches all 4."""
        if bq is None:
            zv, ov = zsb, outt
            osl = slice(h * 4, (h + 1) * 4)
            eng = nc.sync
        elif isinstance(bq, tuple):
            b0, b1 = bq
            zv, ov = zsb[:, b0:b1], outt[:, b0:b1]
            osl = slice(h * 4 + b0, h * 4 + b1)
            eng = nc.sync if b0 % 4 == 0 else nc.scalar
        else:
            zv, ov = zsb[:, bq:bq + 1], outt[:, bq:bq + 1]
            osl = slice(h * 4 + bq, h * 4 + bq + 1)
            eng = nc.sync if bq % 2 == 0 else nc.scalar
        for hw_ in (8, 4, 2):
            nc.vector.tensor_add(zv[:, :, :, 0:hw_], zv[:, :, :, 0:hw_],
                                 zv[:, :, :, hw_:2 * hw_])
        nc.vector.tensor_add(ov, zv[:, :, :, 0], zv[:, :, :, 1])
        eng.dma_start(out=out[jsl, osl, :], in_=ov)

    # ---- halves: s3 phase (h0's interleaves stage-1 columns: s1 is
    # ACT-bound exp while s3 is PE-bound, so they fill each other), then
    # s4+s6 interleaved (lag 2). The last DEFER0 s6 chunks of half 0 drain
    # inside half 1's s3 phase; half 1 runs everything inline.
    COL_LAG = 2                 # s1 columns finished ahead of s3's use
    for jc in range(COL_LAG):
        for st in s1_col_steps(jc):
            st()
    DEFER0 = 5
    deferred = []
    for h in range(2):
        for j in range(NT):
            col = j + COL_LAG
            fill = s1_col_steps(col) if (h == 0 and col < NT) else ()
            s3_j(h, j, fill)
            if h == 0 and j == NT - 1:
                load_s6_inputs()
            if deferred and j >= 1 and (j - 1) % 3 == 0:
                s6_j(*deferred.pop(0))  # drain deferred s6s of previous half
        s4_j(h, 0)
        s4_j(h, 1)
        ndef = DEFER0 if h == 0 else 0
        for j in range(NT):
            if j + 2 < NT:
                s4_j(h, j + 2)
            if j >= NT - ndef:
                deferred.append((h, j))
            else:
                s6_j(h, j)
    for d in deferred:
        s6_j(*d, tail=True)

    global _DBG_TILES
    _DBG_TILES = {"x12t": x12t, "m0": m_col[0], "rinv": rinv}


_DBG_TILES = None


_NC_CACHE = None


def kernel(x, node_embedding, weights_pool, bias_pool):
    global _NC_CACHE
    if _NC_CACHE is None:
        _NC_CACHE = build_nc()
    nc = _NC_CACHE

    import ml_dtypes
    bf16 = ml_dtypes.bfloat16
    f8 = ml_dtypes.float8_e4m3

    x = np.asarray(x, dtype=np.float32)
    E = np.asarray(node_embedding, dtype=np.float32)
    Wp = np.asarray(weights_pool, dtype=np.float32)
    bp = np.asarray(bias_pool, dtype=np.float32)

    etf = np.ascontiguousarray(E.T)
    eth = etf.astype(bf16).astype(np.float32)
    elo = (etf - eth).astype(np.float32)
    et = np.ascontiguousarray(np.concatenate([eth, eth, elo], axis=0)).astype(bf16)
    etlo = np.ascontiguousarray(np.concatenate([eth, elo, eth], axis=0)).astype(bf16)
    # Chebyshev host fold: T2 = 2 S^2 - I  =>  k0' = W0 - W2, k2' = 2 W2
    Wp = Wp.copy()
    Wp[:, 0] -= Wp[:, 2]
    Wp[:, 2] *= 2.0
    # wpf[(k,i), (o,d)] = Wp[d,k,i,o]; pass A = [k1; k2] bf16,
    # pass B = [k0; bias] as fp8 hi/lo DoubleRow operand
    wpf = np.ascontiguousarray(Wp.transpose(1, 2, 3, 0).reshape(CHEB_K * C, DO))
    wpfa = np.ascontiguousarray(wpf[64:192]).astype(bf16)
    w0 = wpf[0:64]
    w0h = w0.astype(f8)
    w0l = (w0 - w0h.astype(np.float32)).astype(f8)
    bprow = bp.T.reshape(1, DO).astype(f8)
    w8t0 = np.concatenate([w0h, w0h], axis=0)          # [128, DO]
    w8t1 = np.zeros((128, DO), dtype=f8)
    w8t1[0:64] = w0l
    w8t1[64:65] = bprow
    w8 = np.ascontiguousarray(
        np.stack([w8t0, w8t1], axis=1).reshape(128, 2 * DO))
    en_b = np.ascontiguousarray(E).astype(bf16)

    in_maps = []
    for c in range(NCORES):
        xc = x[BC * c:BC * (c + 1)]
        xct = xc.transpose(2, 0, 1).reshape(C, BC * N)  # [C, (b,n)] fp32
        xhi = xct.astype(f8)
        xlo = (xct - xhi.astype(np.float32)).astype(f8)
        x8t0 = np.concatenate([xhi, xlo], axis=0)       # [128, BC*N]
        x8t1 = np.zeros((128, BC * N), dtype=f8)
        x8t1[0:64] = xhi
        x8t1[64:65] = 1.0
        xf8 = np.ascontiguousarray(
            np.stack([x8t0, x8t1], axis=1).reshape(128, 2 * BC * N))
        x2 = xc.transpose(1, 0, 2).reshape(N, F).astype(bf16)
        in_maps.append({
            "x2a": np.ascontiguousarray(x2[:, 0:FH]),
            "x2b": np.ascontiguousarray(x2[:, FH:F]),
            "xf8": xf8,
            "et": et, "etlo": etlo, "en": en_b, "wpfa": wpfa, "w8": w8,
        })
    res = run_bass_kernel_spmd(nc, in_maps, list(range(NCORES)))
    return np.concatenate(
        [res.results[c]["out"].astype(np.float32).transpose(1, 0, 2)
         for c in range(NCORES)], axis=0)


if __name__ == "__main__":
    rng = np.random.default_rng(0)
    inputs = {
        "x": rng.standard_normal((B, N, C), dtype=np.float32),
        "node_embedding": rng.standard_normal((N, EMBED), dtype=np.float32),
        "weights_pool": (rng.standard_normal((EMBED, CHEB_K, C, C), dtype=np.float32) * 0.1),
        "bias_pool": (rng.standard_normal((EMBED, C), dtype=np.float32) * 0.1),
    }
    got = kernel(**inputs)
    print("out", got.shape, got.dtype, np.abs(got).max())



# revision 111
# speedup vs baseline: 1.0002x; 1.0002x over previous
"""AVWGCN Trainium2 kernel: adaptive-adjacency Chebyshev GCN.

Math (per core, batch-sharded over B: 8 batches/core):
  A = relu(E @ E^T) (symmetric), M = exp(A), r = rowsum(M), S = diag(1/r) M
  X2[m,(b,c)] = x[b,m,c]
  x1 = diag(1/r) (M @ X2)            (T1 term)
  x2o = diag(1/r) (M @ x1)           (= S^2 x; T2 = 2 S^2 - I folded on host)
  out[b,n,o] = sum_d E[n,d] * ( sum_{k,i} xg_k[n,(b,i)] Wp'[d,k,i,o] + bp[d,o] )
    with Wp'_0 = Wp_0 - Wp_2, Wp'_2 = 2 Wp_2 (host fold of the Chebyshev -x
    term), so xg = [x, S x, S^2 x].
Key structure:
  - stage 1 computes M COLUMN-block by column-block into per-column tiles:
    column j is exactly the lhsT set s3(j)/s4(j) contract with, so the
    ACT-heavy exp pipeline overlaps s3's PE-heavy matmuls with clean
    per-tile dependencies (one stage-1 column finishes COL_LAG iterations
    ahead of its s3 use; steps are interleaved one per s3 matmul to avoid
    head-of-line blocking of the in-order PE queue).
  - row sums ride s3's h0 matmul as an extra ones column of the rhs
    (M @ [X2|1]); reciprocal per column feeds both halves' scales.
  - x1/x2o written interleaved into a padded tile xp[j][n, (b,128)] (x1 at
    cols b*128+c, x2o at b*128+64+c); ONE hw DMA transpose per (half, j)
    lands [x1^T; x2^T] stacked on 128 partitions = the K=128 lhsT (k1;k2).
  - gconv via Z-form: Z[n,(o,d)] = bf16 [x1;x2]^T-pass + ONE fp8 DoubleRow
    pass (0.5 cyc/col) carrying the k0 term exactly via a host-side
    e4m3 hi/lo split (xhi*W0hi + xlo*W0hi + xhi*W0lo; the dropped lo*lo
    term is ~0.1% of the k0 term) with bias on a ones row; epilogue: ACT
    casts Z->bf16, DVE multiplies by broadcast E and tree-reduces d.
  - batch dim processed in two halves; the last DEFER0 epilogues of half 0
    drain inside half 1's s3 phase (whose PE-bound window otherwise leaves
    ACT/DVE idle).
"""

from contextlib import ExitStack

import numpy as np

import concourse.bass as bass
import concourse.mybir as mybir
import concourse.tile as tile
from concourse.bass_utils import run_bass_kernel_spmd

B, N, C, CHEB_K, EMBED = 64, 2048, 64, 3, 16
NCORES = 8
BC = B // NCORES            # batches per core
F = BC * C                  # 512: free width of X2 [m, (b,c)]
FH = F // 2                 # 256: per-batch-half width
NT = N // 128               # 16 n-chunks
FP32 = mybir.dt.float32
BF16 = mybir.dt.bfloat16
FP8 = mybir.dt.float8e4
MM_DT = BF16
DO = C * EMBED              # 1024, Z free width, (o, d) ordered


_WAIT_CAP = {"InstDMACopy": 1}
_WAIT_SAFE = {"InstEventSemaphore", "InstCall",
              "InstUnconditionalBranch", "InstISA", "InstRegisterMove"}


def _split_excess_waits(nc):
    """Walrus rejects compute instructions carrying more sync waits than the
    ISA struct can encode. Hoist excess waits onto an inserted same-engine
    Drain immediately before the instruction (semantically identical)."""
    SyncInfo = None
    n_fix = 0
    for f in nc.m.functions:
        for blk in f.blocks:
            out_insts = []
            for inst in blk.instructions:
                tn = type(inst).__name__
                si = inst.sync_info
                w = list(si.on_wait) if (si is not None and si.on_wait) else []
                cap = _WAIT_CAP.get(tn, 1)
                if tn not in _WAIT_SAFE and len(w) > cap:
                    if SyncInfo is None:
                        SyncInfo = type(si)
                    for wx in w:
                        d = mybir.InstDrain(name=f"I-wsplit{nc.next_id()}",
                                            ins=[], outs=[])
                        d.engine = inst.engine
                        d.sync_info = SyncInfo(on_wait=[wx], on_update=[])
                        out_insts.append(d)
                    si.on_wait = []
                    n_fix += 1
                out_insts.append(inst)
            blk.instructions[:] = out_insts
    return n_fix


def build_nc():
    nc = bass.Bass()
    x2a = nc.dram_tensor("x2a", [N, FH], BF16, kind="ExternalInput").ap()
    x2b = nc.dram_tensor("x2b", [N, FH], BF16, kind="ExternalInput").ap()
    xf8_d = nc.dram_tensor("xf8", [128, 2 * BC * N], FP8, kind="ExternalInput").ap()
    et = nc.dram_tensor("et", [3 * EMBED, N], BF16, kind="ExternalInput").ap()
    etlo_d = nc.dram_tensor("etlo", [3 * EMBED, N], BF16, kind="ExternalInput").ap()
    en = nc.dram_tensor("en", [N, EMBED], BF16, kind="ExternalInput").ap()
    wpfa = nc.dram_tensor("wpfa", [128, DO], BF16, kind="ExternalInput").ap()
    w8_d = nc.dram_tensor("w8", [128, 2 * DO], FP8, kind="ExternalInput").ap()
    # device out is [n, b, c] (matching the SBUF epilogue layout): the DMA
    # gets 512B full-rate runs and 4x fewer descriptors; host transposes
    out = nc.dram_tensor("out", [N, BC, C], BF16, kind="ExternalOutput").ap()

    with tile.TileContext(nc) as tc:
        with ExitStack() as ctx:
            kernel_body(ctx, tc, out, x2a, x2b, xf8_d, et, etlo_d, en, wpfa, w8_d)
    _split_excess_waits(nc)
    return nc


def kernel_body(ctx, tc, out, x2a, x2b, xf8_d, et, etlo_d, en, wpfa, w8_d):
    nc = tc.nc

    singles = ctx.enter_context(tc.tile_pool(name="singles", bufs=1))
    zsb_pool = ctx.enter_context(tc.tile_pool(name="zsb", bufs=3))
    outs_pool = ctx.enter_context(tc.tile_pool(name="outs", bufs=2))

    # ---- constants / inputs ----
    # split-precision E^T, K-stacked: one K=48 matmul computes
    # EhiEhi^T + EhiElo^T + EloEhi^T (lhsT=[Ehi;Ehi;Elo], rhs=[Ehi;Elo;Ehi])
    ethi = singles.tile([3 * EMBED, N], MM_DT, tag="ethi")
    etlo = singles.tile([3 * EMBED, N], MM_DT, tag="etlo")
    # split first chunks so the first stage-1 matmul starts ASAP
    nc.sync.dma_start(out=ethi, in_=et)
    nc.sync.dma_start(out=etlo[:, 0:512], in_=etlo_d[:, 0:512])
    nc.sync.dma_start(out=etlo[:, 512:N], in_=etlo_d[:, 512:N])
    wa_sb = singles.tile([128, DO], MM_DT, tag="wa")   # rows: [k1; 2*k2]
    # fp8 DoubleRow operands for the k0+bias pass (host-prepared hi/lo split):
    #   k-tile 0 rows = [xhi(64); xlo(64)], k-tile 1 rows = [xhi(64); ones; 0]
    #   paired with   [W0hi; W0hi]         and          [W0lo; bias; junk]
    xf8 = singles.tile([128, 2, BC * N], FP8, tag="xf8")
    w8_sb = singles.tile([128, 2, DO], FP8, tag="w8")
    # E chunks for the epilogue: en_sb[p, j, d] = E[j*128+p, d]
    en_sb = singles.tile([128, NT, EMBED], MM_DT, tag="en_sb")
    # x12t: [x1^T; x2^T] stacked on partitions, filled by DMA transposes
    x12t = singles.tile([128, BC * N], MM_DT, tag="x12t")
    x12t_v = x12t.rearrange("p (b n) -> p b n", b=BC)

    # ---- stage 1: M = exp(relu(E E^T)), computed COLUMN-block by
    # column-block into per-column tiles: column j is exactly the lhsT set
    # s3(j)/s4(j) contract with, so each s3(j) depends on one tile that was
    # finished COL_LAG iterations earlier — no cross-phase false deps, no
    # transposes, and stage 1's ACT-heavy exp overlaps s3's PE-heavy matmuls.
    m_col = [singles.tile([128, NT, 128], MM_DT, name=f"mc{j}", tag=f"mc{j}")
             for j in range(NT)]

    def m_ap(a, j):
        """lhsT for contraction chunk a, output chunk j: M[a-rows, j-cols]."""
        return m_col[j][:, a, :]

    ps_mm = ctx.enter_context(tc.tile_pool(name="ps_mm", bufs=2, space="PSUM"))
    ps_z = ctx.enter_context(tc.tile_pool(name="ps_z", bufs=3, space="PSUM"))

    def s1_col_steps(j):
        """Closures (one per matmul) for column-block j of stage 1:
        M[:, jsl] = max(exp(E E^T), 1) in two 8-row-chunk groups. Interleaved
        between s3 matmuls so the PSUM-ring wait on the ACT exp never
        head-of-line-blocks the in-order PE queue."""
        jsl = slice(j * 128, (j + 1) * 128)
        state = {}

        def step(g, i):
            a = g * 8 + i

            def run():
                if i == 0:
                    state["pa"] = ps_z.tile([128, DO], FP32, name="pa", tag="pz")
                pa = state["pa"]
                nc.tensor.matmul(pa[:, i * 128:(i + 1) * 128],
                                 lhsT=ethi[:, a * 128:(a + 1) * 128],
                                 rhs=etlo[:, jsl], start=True, stop=True)
                if i == 7:
                    dst = m_col[j][:, g * 8:(g + 1) * 8, :]
                    nc.scalar.activation(
                        out=dst, in_=pa,
                        func=mybir.ActivationFunctionType.Exp)
                    nc.vector.tensor_scalar_max(dst, dst, 1.0)
            return run

        return [step(g, i) for g in range(2) for i in range(8)]

    # ---- stage 2: X2 halves (h0 includes a ones column at FH for the row
    # sums; see s3), split into two tiles (a 0-7 / 8-15) so the chunked
    # loads have no same-tile WAW chain.
    FW = 2 * FH + 1             # 513: [h0 | ones col | h1]
    x2big2 = [singles.tile([128, NT // 2, FW], MM_DT, name=f"x2big{g}",
                           tag=f"x2big{g}")
              for g in range(2)]
    x2ar = x2a.rearrange("(a p) f -> p a f", a=NT)
    x2br = x2b.rearrange("(a p) f -> p a f", a=NT)
    for g in range(2):
        asl = slice(g * 8, (g + 1) * 8)
        nc.vector.memset(x2big2[g][:, :, FH:FH + 1], 1.0)
        nc.gpsimd.dma_start(out=x2big2[g][:, :, 0:FH], in_=x2ar[:, asl, :])
        nc.gpsimd.dma_start(out=x2big2[g][:, :, FH + 1:FW],
                            in_=x2br[:, asl, :])

    def x2_ap(a, hsl):
        return x2big2[a // 8][:, a % 8, hsl]
    def load_s6_inputs():
        """Issued after the h0 s3 loop: these are first used ~halfway in,
        and issuing them early would contend the (exclusive) DMA engine
        device against the latency-critical mirror transposes."""
        nc.gpsimd.dma_start(out=wa_sb, in_=wpfa)
        xf8v = xf8.rearrange("p t n -> p (t n)")
        for g in range(8):
            csl = slice(g * 4096, (g + 1) * 4096)
            nc.gpsimd.dma_start(out=xf8v[:, csl], in_=xf8_d[:, csl])
        nc.gpsimd.dma_start(out=w8_sb.rearrange("p t n -> p (t n)"), in_=w8_d)
        nc.gpsimd.dma_start(out=en_sb,
                            in_=en.rearrange("(j p) d -> p j d", j=NT))
    rinv = singles.tile([128, NT], FP32, tag="rinv")

    # xp[j]: padded per-half [n, (b4, 128)]: x1 at cols b*128+c, x2o at +64
    xp = [singles.tile([128, 4, 128], MM_DT, name=f"xp{j}", tag=f"xp{j}")
          for j in range(NT)]

    def s3_j(h, j, fill=()):
        """x1[:, h-half] = diag(1/r) M X2[:, h-half] -> xp[j] cols b*128+c.
        `fill` steps (stage-1 chunks) are issued one per s3 matmul."""
        jsl = slice(j * 128, (j + 1) * 128)
        w = FH + 1 if h == 0 else FH
        hsl = slice(0, FH + 1) if h == 0 else slice(FH + 1, FW)
        fill = list(fill)
        pm = ps_mm.tile([128, FH + 1], FP32, tag="pm")
        for a in range(NT):
            nc.tensor.matmul(pm[:, 0:w], lhsT=m_ap(a, j),
                             rhs=x2_ap(a, hsl),
                             start=(a == 0), stop=(a == NT - 1))
            if a % 2 == 1:
                for _ in range(min(2, len(fill))):
                    fill.pop(0)()
        if h == 0:
            # rsum rode along as the last column (M @ ones)
            nc.vector.reciprocal(out=rinv[:, j:j + 1], in_=pm[:, FH:FH + 1])
        nc.scalar.activation(out=xp[j][:, :, 0:C], in_=pm[:, 0:FH],
                             func=mybir.ActivationFunctionType.Copy,
                             scale=rinv[:, j:j + 1])
        for f in fill:
            f()

    def s4_j(h, j):
        """x2o = diag(1/r) M x1 -> xp[j] cols b*128+64+c, then transpose."""
        jsl = slice(j * 128, (j + 1) * 128)
        pm = ps_mm.tile([128, FH + 1], FP32, tag="pm")
        for a in range(NT):
            nc.tensor.matmul(pm[:, 0:FH], lhsT=m_ap(a, j),
                             rhs=xp[a][:, :, 0:C],
                             start=(a == 0), stop=(a == NT - 1))
        nc.scalar.activation(out=xp[j][:, :, C:128], in_=pm[:, 0:FH],
                             func=mybir.ActivationFunctionType.Copy,
                             scale=rinv[:, j:j + 1])
        # one XBAR transpose: 4 blocks of 128 cols -> [x1^T; x2^T] on 128
        # partitions, into the 4 b-column ranges of x12t for this half
        nc.sync.dma_start_transpose(
            out=x12t_v[:, h * 4:(h + 1) * 4, jsl],
            in_=xp[j].rearrange("p b q -> p (b q)"))

    def s6_j(h, j, tail=False, pair=False):
        """Z matmuls + epilogue for the 4 batches of half h, chunk j."""
        jsl = slice(j * 128, (j + 1) * 128)
        # E[jsl] broadcast over (4 batches, C outputs) via zero-stride dims
        erep_bc = bass.AP(tensor=en_sb.tensor, offset=en_sb.offset + j * EMBED,
                          ap=[en_sb.ap[0], [0, 4], [0, C], [1, EMBED]])

        zsb = zsb_pool.tile([128, 4, C, EMBED], MM_DT, tag="zsb")
        outt = outs_pool.tile([128, 4, C], MM_DT, tag="outt")
        for bq in range(4):
            b = h * 4 + bq
            col = b * N + j * 128
            pz = ps_z.tile([128, DO], FP32, tag="pz")
            for hh in range(2):
                sl = slice(hh * 512, (hh + 1) * 512)
                nc.tensor.matmul(pz[:, sl], lhsT=x12t[:, col:col + 128],
                                 rhs=wa_sb[:, sl], start=True, stop=False)
                nc.tensor.matmul(pz[:, sl], lhsT=xf8[:, :, col:col + 128],
                                 rhs=w8_sb[:, :, sl], start=False, stop=True,
                                 perf_mode=mybir.MatmulPerfMode.DoubleRow)
            if pair and bq == 0:
                nc.gpsimd.tensor_copy(
                    out=zsb[:, bq].rearrange("p o d -> p (o d)"), in_=pz)
            else:
                nc.scalar.copy(
                    out=zsb[:, bq].rearrange("p o d -> p (o d)"), in_=pz)
            if tail:
                zv = zsb[:, bq:bq + 1]
                ebc = bass.AP(tensor=erep_bc.tensor, offset=erep_bc.offset,
                              ap=[erep_bc.ap[0], [0, 1], [0, C], [1, EMBED]])
                nc.vector.tensor_mul(zv, zv, ebc)
                _epilogue_tree(zsb, outt, h, j, jsl, bq)
        if not tail:
            nc.vector.tensor_mul(zsb, zsb, erep_bc)  # in-place *E (bf16 2x)
            _epilogue_tree(zsb, outt, h, j, jsl, None)

    def _epilogue_tree(zsb, outt, h, j, jsl, bq):
        """d-tree-reduce (DVE for the big levels, Pool for the small) and
        the out DMA; bq=None batches all 4."""
        if bq is None:
            zv, ov = zsb, outt
            osl = slice(h * 4, (h + 1) * 4)
            eng = nc.sync
        elif isinstance(bq, tuple):
            b0, b1 = bq
            zv, ov = zsb[:, b0:b1], outt[:, b0:b1]
            osl = slice(h * 4 + b0, h * 4 + b1)
            eng = nc.sync if b0 % 4 == 0 else nc.scalar
        else:
            zv, ov = zsb[:, bq:bq + 1], outt[:, bq:bq + 1]
            osl = slice(h * 4 + bq, h * 4 + bq + 1)
            eng = nc.sync if bq % 2 == 0 else nc.scalar
        for hw_ in (8, 4, 2):
            nc.vector.tensor_add(zv[:, :, :, 0:hw_], zv[:, :, :, 0:hw_],
                                 zv[:, :, :, hw_:2 * hw_])
        nc.vector.tensor_add(ov, zv[:, :, :, 0], zv[:, :, :, 1])
        eng.dma_start(out=out[jsl, osl, :], in_=ov)

    # ---- halves: s3 phase (h0's interleaves stage-1 columns: s1 is
    # ACT-bound exp while s3 is PE-bound, so they fill each other), then
    # s4+s6 interleaved (lag 2). The last DEFER0 s6 chunks of half 0 drain
    # inside half 1's s3 phase; half 1 runs everything inline.
    COL_LAG = 2                 # s1 columns finished ahead of s3's use
    for jc in range(COL_LAG):
        for st in s1_col_steps(jc):
            st()
    DEFER0 = 5
    deferred = []
    for h in range(2):
        for j in range(NT):
            col = j + COL_LAG
            fill = s1_col_steps(col) if (h == 0 and col < NT) else ()
            s3_j(h, j, fill)
            if h == 0 and j == NT - 1:
                load_s6_inputs()
            if deferred and j >= 1 and (j - 1) % 3 == 0:
                s6_j(*deferred.pop(0))  # drain deferred s6s of previous half
        s4_j(h, 0)
        s4_j(h, 1)
        ndef = DEFER0 if h == 0 else 0
        for j in range(NT):
            if j + 2 < NT:
                s4_j(h, j + 2)
            if j >= NT - ndef:
                deferred.append((h, j))
            else:
                s6_j(h, j)
    for d in deferred:
        s6_j(*d, tail=True)

    global _DBG_TILES
    _DBG_TILES = {"x12t": x12t, "m0": m_col[0], "rinv": rinv}


_DBG_TILES = None


_NC_CACHE = None


def kernel(x, node_embedding, weights_pool, bias_pool):
    global _NC_CACHE
    if _NC_CACHE is None:
        _NC_CACHE = build_nc()
    nc = _NC_CACHE

    import ml_dtypes
    bf16 = ml_dtypes.bfloat16
    f8 = ml_dtypes.float8_e4m3

    x = np.asarray(x, dtype=np.float32)
    E = np.asarray(node_embedding, dtype=np.float32)
    Wp = np.asarray(weights_pool, dtype=np.float32)
    bp = np.asarray(bias_pool, dtype=np.float32)

    etf = np.ascontiguousarray(E.T)
    eth = etf.astype(bf16).astype(np.float32)
    elo = (etf - eth).astype(np.float32)
    et = np.ascontiguousarray(np.concatenate([eth, eth, elo], axis=0)).astype(bf16)
    etlo = np.ascontiguousarray(np.concatenate([eth, elo, eth], axis=0)).astype(bf16)
    # Chebyshev host fold: T2 = 2 S^2 - I  =>  k0' = W0 - W2, k2' = 2 W2
    Wp = Wp.copy()
    Wp[:, 0] -= Wp[:, 2]
    Wp[:, 2] *= 2.0
    # wpf[(k,i), (o,d)] = Wp[d,k,i,o]; pass A = [k1; k2] bf16,
    # pass B = [k0; bias] as fp8 hi/lo DoubleRow operand
    wpf = np.ascontiguousarray(Wp.transpose(1, 2, 3, 0).reshape(CHEB_K * C, DO))
    wpfa = np.ascontiguousarray(wpf[64:192]).astype(bf16)
    w0 = wpf[0:64]
    w0h = w0.astype(f8)
    w0l = (w0 - w0h.astype(np.float32)).astype(f8)
    bprow = bp.T.reshape(1, DO).astype(f8)
    w8t0 = np.concatenate([w0h, w0h], axis=0)          # [128, DO]
    w8t1 = np.zeros((128, DO), dtype=f8)
    w8t1[0:64] = w0l
    w8t1[64:65] = bprow
    w8 = np.ascontiguousarray(
        np.stack([w8t0, w8t1], axis=1).reshape(128, 2 * DO))
    en_b = np.ascontiguousarray(E).astype(bf16)

    in_maps = []
    for c in range(NCORES):
        xc = x[BC * c:BC * (c + 1)]
        xct = xc.transpose(2, 0, 1).reshape(C, BC * N)  # [C, (b,n)] fp32
        xhi = xct.astype(f8)
        xlo = (xct - xhi.astype(np.float32)).astype(f8)
        x8t0 = np.concatenate([xhi, xlo], axis=0)       # [128, BC*N]
        x8t1 = np.zeros((128, BC * N), dtype=f8)
        x8t1[0:64] = xhi
        x8t1[64:65] = 1.0
        xf8 = np.ascontiguousarray(
            np.stack([x8t0, x8t1], axis=1).reshape(128, 2 * BC * N))
        x2 = xc.transpose(1, 0, 2).reshape(N, F).astype(bf16)
        in_maps.append({
            "x2a": np.ascontiguousarray(x2[:, 0:FH]),
            "x2b": np.ascontiguousarray(x2[:, FH:F]),
            "xf8": xf8,
            "et": et, "etlo": etlo, "en": en_b, "wpfa": wpfa, "w8": w8,
        })
    res = run_bass_kernel_spmd(nc, in_maps, list(range(NCORES)))
    return np.concatenate(
        [res.results[c]["out"].astype(np.float32).transpose(1, 0, 2)
         for c in range(NCORES)], axis=0)


if __name__ == "__main__":
    rng = np.random.default_rng(0)
    inputs = {
        "x": rng.standard_normal((B, N, C), dtype=np.float32),
        "node_embedding": rng.standard_normal((N, EMBED), dtype=np.float32),
        "weights_pool": (rng.standard_normal((EMBED, CHEB_K, C, C), dtype=np.float32) * 0.1),
        "bias_pool": (rng.standard_normal((EMBED, C), dtype=np.float32) * 0.1),
    }
    got = kernel(**inputs)
    print("out", got.shape, got.dtype, np.abs(got).max())

